# revision 16
# baseline (speedup 1.0000x reference)
"""ECGEConv (relational graph conv) Trainium2 kernel, 8-core SPMD.

Strategy (row-sharded, aggregate-then-transform):
  - Host prep (index math only): in-degree via bincount,
    norm = rsqrt(deg)[row]*rsqrt(deg)[col]*w, edges routed to the core
    owning their destination row, bucketed by (dest row block, column
    half, relation type), padded to a uniform static schedule shared by
    all 8 cores; x is passed as fp16.
  - Each core owns an N/8-row output slice; output slices are disjoint
    so no inter-core collectives are needed.  Per 128-edge unit:
      gpsimd: batched dma_gather (MoE SWDGE ucode, int16 indices local
              to a 25k-column half, <=8 units = 1024 rows per op -- the
              ucode descriptor ring caps one op at 1024 rows -- spread
              over 4 SWDGE queues, honest per-group row counts; gather
              buffers are zeroed once at startup on DVE) -> X_g fp16
      DVE:    one-hot scatter matrices built in bulk per gather group
              via two tensor_tensor ops with stride-0 broadcast APs:
              mask = (iota_r == lrow), pm = mask * wn, laid out
              [edge, unit, row] so each unit's matrix is contiguous
              for the PE rhs stream
      PE:     psum_agg[fi, (t,h)*128+r] += X_g^T @ pm[:, unit, :]
              (scatter via matmul, fp32 accumulation)
    Per 128-row block: ACT copies psum_agg -> SBUF (fp16); PE applies
    the four relation matrices W_t (fp16) plus a rank-1 ones x bias
    matmul -> psum_out[r, fo]; ACT LeakyReLU(0.01) reads the psum
    directly; HWDGE DMA writes the contiguous output rows.
  - Measured (reps-difference, pipelined dispatch, 8 cores): ~244 us
    HW per pass (E=600k, N=50k, D=128); max rel err ~3.6e-4 vs the
    fp32 reference (fp16 gather/weights precision).
"""
import json
import sys

sys.path.insert(0, "/opt/trn_rl_repo")

import numpy as np

import concourse.bass as bass
import concourse.bacc as bacc
import concourse.mybir as mybir

NCORES = 8
NTYPES = 4
DIN = 128
DOUT = 128
P = 128

_DIMS = {"N": 50000}
_ACT = {"func": "Lrelu"}
_GDT = {"np": "float16", "my": "float16"}  # gather dtype
_NQ = {"n": 1}  # SWDGE queues to spread gathers over (indirect mode)
_GMODE = {"gcap": 16, "nq": 4, "sp": True, "nbuf": 8, "npb": 3}


def _half():
    return (_DIMS["N"] + 1) // 2


def _rows_per_core():
    return _DIMS["N"] // NCORES


def _nblk():
    return (_rows_per_core() + P - 1) // P


# ---------------------------------------------------------------------------
# Walrus in this toolchain rejects >1 semaphore wait per instruction; move
# excess waits onto Drain carrier instructions at the BIR-JSON level.
# ---------------------------------------------------------------------------
_sync_split_installed = False


def _spread_queues_json(block, counter, nq):
    for inst in block.get("instructions") or []:
        if (inst.get("opcode") == "DMACopy"
                and inst.get("queue") == "qPoolDynamic" and nq > 1):
            q = counter[1] % nq
            counter[1] += 1
            if q:
                inst["queue"] = f"qPoolDynamic{q}"
    for sb in block.get("blocks") or []:
        _spread_queues_json(sb, counter, nq)


def _split_block_json(block, counter):
    insts = block.get("instructions")
    if insts:
        new_insts = []
        for inst in insts:
            si = inst.get("sync_info")
            if si:
                waits = si.get("on_wait") or []
                if len(waits) > 1:
                    excess, keep = waits[:-1], waits[-1:]
                    for w in excess:
                        counter[0] += 1
                        new_insts.append({
                            "opcode": "Drain",
                            "engine": inst["engine"],
                            "name": f"SWS-{counter[0]}",
                            "ins": [], "outs": [],
                            "debug": inst.get("debug", 0),
                            "sync_info": {"on_wait": [w], "on_update": []},
                        })
                    si["on_wait"] = keep
            new_insts.append(inst)
        block["instructions"] = new_insts
    for sb in block.get("blocks") or []:
        _split_block_json(sb, counter)


def _install_sync_split():
    global _sync_split_installed
    if _sync_split_installed:
        return
    from concourse import bass2jax

    orig = bass2jax.compile_bir_kernel

    def patched(bir_json, tmpdir, neff_name="file.neff"):
        d = json.loads(bir_json)
        counter = [0, 0]
        for fn in d.get("functions", []):
            for b in fn.get("blocks", []):
                _split_block_json(b, counter)
                _spread_queues_json(b, counter, _NQ["n"])
        return orig(json.dumps(d).encode(), tmpdir, neff_name=neff_name)

    bass2jax.compile_bir_kernel = patched
    _sync_split_installed = True


# ---------------------------------------------------------------------------
# Host-side prep: degree/norm, sharding, bucketing, static schedule.
# ---------------------------------------------------------------------------
def _prepare(edge_index, edge_type, edge_weight):
    N = _DIMS["N"]
    rpc = _rows_per_core()
    nblk = _nblk()

    row = np.asarray(edge_index[0], dtype=np.int64)
    col = np.asarray(edge_index[1], dtype=np.int64)
    et = np.asarray(edge_type, dtype=np.int64)
    ew = np.asarray(edge_weight, dtype=np.float32)

    deg = np.bincount(col, minlength=N).astype(np.float32)
    dis = np.zeros(N, dtype=np.float32)
    nz = deg > 0
    dis[nz] = 1.0 / np.sqrt(deg[nz])
    norm = (dis[row] * dis[col] * ew).astype(np.float32)

    core = row // rpc
    lrow = row - core * rpc
    blk = lrow // P
    rloc = lrow - blk * P

    HALF = _half()
    half = (col >= HALF).astype(np.int64)
    order = np.lexsort((col, half, et, blk, core))
    core_s, blk_s, et_s = core[order], blk[order], et[order]
    half_s = half[order]
    col_s, rloc_s, norm_s = col[order], rloc[order], norm[order]

    counts = np.zeros((NCORES, nblk, NTYPES, 2), dtype=np.int64)
    np.add.at(counts, (core_s, blk_s, et_s, half_s), 1)
    units_bth = (counts.max(axis=0) + P - 1) // P          # [nblk, NTYPES, 2]
    # guarantee >=1 unit per (b, t) so the psum slice is always written
    bt_tot = units_bth.sum(axis=2)
    units_bth[:, :, 0] = np.maximum(units_bth[:, :, 0], (bt_tot == 0))
    T = int(units_bth.sum())

    gidx = np.zeros((NCORES, P, T), dtype=np.int32)
    lrow_t = np.zeros((NCORES, P, T), dtype=np.float32)
    w_t = np.zeros((NCORES, P, T), dtype=np.float32)
    cnts_u = np.zeros((NCORES, T), dtype=np.int64)   # real edges per unit

    starts = np.cumsum(counts.reshape(-1)).reshape(counts.shape) - counts

    schedule = []   # (b, t, h, nu, first_of_bt, last_of_bt)
    ucol = 0
    for b in range(nblk):
        seen = {t: 0 for t in range(NTYPES)}
        h_order = (0, 1)
        for h in h_order:
            for t in range(NTYPES):
                if units_bth[b, t, h] == 0:
                    continue
                nu = int(units_bth[b, t, h])
                seen[t] += 1
                nh = int((units_bth[b, t, :] > 0).sum())
                schedule.append((b, t, h, nu, seen[t] == 1, seen[t] == nh))
                for c in range(NCORES):
                    s = int(starts[c, b, t, h])
                    cnt = int(counts[c, b, t, h])
                    room = nu * P
                    assert cnt <= room
                    g = np.zeros(room, dtype=np.int32)
                    lr = np.zeros(room, dtype=np.float32)
                    wv = np.zeros(room, dtype=np.float32)
                    if cnt > 0:
                        g[:cnt] = col_s[s:s + cnt]
                        g[cnt:] = col_s[s + cnt - 1]
                        lr[:cnt] = rloc_s[s:s + cnt]
                        wv[:cnt] = norm_s[s:s + cnt]
                    gidx[c, :, ucol:ucol + nu] = g.reshape(nu, P).T
                    lrow_t[c, :, ucol:ucol + nu] = lr.reshape(nu, P).T
                    w_t[c, :, ucol:ucol + nu] = wv.reshape(nu, P).T
                    full, rem = divmod(cnt, P)
                    for j in range(nu):
                        cnts_u[c, ucol + j] = (
                            P if j < full else (rem if j == full else 0))
                ucol += nu
    assert ucol == T
    return schedule, T, gidx, lrow_t, w_t, cnts_u


def _gather_groups(schedule, gcap):
    """Group consecutive same-half units into dma_gather ops (<=gcap units)."""
    groups = []   # (h, u_start, nu_g)
    u = 0
    for (b, t, h, nu, _f, _l) in schedule:
        j = 0
        while j < nu:
            if groups and groups[-1][0] == h and groups[-1][2] < gcap \
                    and groups[-1][1] + groups[-1][2] == u + j:
                gh, gu, gn = groups.pop()
                take = min(gcap - gn, nu - j)
                groups.append((gh, gu, gn + take))
                j += take
            else:
                take = min(gcap, nu - j)
                groups.append((h, u + j, take))
                j += take
        u += nu
    return groups


def _pack_idx16(schedule, groups, gidx, cnts_u, full_first=None):
    """Per-core int16 index table + per-group valid counts for dma_gather.
    The first `full_first` groups are packed with full static counts (pad
    slots duplicate a real index); with the gpsimd startup memset of the
    gather buffers this is unnecessary, so it defaults to 0."""
    if full_first is None:
        full_first = 0
    icols = sum(8 * gn for (_h, _u, gn) in groups)
    idx16 = np.zeros((NCORES, P, icols), dtype=np.int16)
    gcnt = np.zeros((NCORES, 1, len(groups)), dtype=np.int32)
    for c in range(NCORES):
        off = 0
        for gi, (h, u0, gn) in enumerate(groups):
            vals = np.full(gn * P, -1, dtype=np.int32)
            for j in range(gn):
                k = int(cnts_u[c, u0 + j])
                if k > 0:
                    v = gidx[c, :k, u0 + j].astype(np.int32) - h * _half()
                    assert v.min() >= 0 and v.max() < 32768
                    vals[j * P:j * P + k] = v
            # valid count = non-negative entries; ucode wants them in order,
            # trailing -1 skipped.  Interior -1 not allowed: compact per unit
            # is already contiguous; but a short unit followed by a full unit
            # leaves interior -1.  Replace interior -1 with duplicate idx.
            if gi < full_first:
                # replace every -1 with a duplicate of a real index
                if (vals >= 0).any():
                    fill = vals[vals >= 0][0]
                    prev = fill
                    for i in range(gn * P):
                        if vals[i] < 0:
                            vals[i] = prev
                        else:
                            prev = vals[i]
                else:
                    vals[:] = 0
                gcnt[c, 0, gi] = gn * P
                packed = vals.astype(np.int16).reshape(gn * 8, 16).T
                idx16[c, :, off:off + gn * 8] = np.tile(packed, (8, 1))
                off += gn * 8
                continue
            nonneg = vals >= 0
            if nonneg.any():
                last = np.max(np.nonzero(nonneg)[0])
                seg = vals[:last + 1]
                if (seg < 0).any():
                    fill = seg[seg >= 0][0]
                    prev = fill
                    for i in range(last + 1):
                        if seg[i] < 0:
                            seg[i] = prev
                        else:
                            prev = seg[i]
                nvalid = last + 1
            else:
                vals[0] = 0
                nvalid = 1
            gcnt[c, 0, gi] = nvalid
            packed = vals.astype(np.int16).reshape(gn * 8, 16).T  # [16, gn*8]
            idx16[c, :, off:off + gn * 8] = np.tile(packed, (8, 1))
            off += gn * 8
    return idx16, gcnt


# ---------------------------------------------------------------------------
# Device program (one program, SPMD across 8 cores)
# ---------------------------------------------------------------------------
def _build_nc(schedule, T, nbuf=None, reps=1):
    gcap = _GMODE["gcap"]
    if nbuf is None:
        nbuf = _GMODE["nbuf"]
    NPB = _GMODE["npb"]
    HALF = _half()
    groups = _gather_groups(schedule, gcap)
    N = _DIMS["N"]
    rpc = _rows_per_core()
    nblk = _nblk()

    nc = bacc.Bacc("TRN2", target_bir_lowering=False, debug=False,
                   enable_asserts=True, num_devices=NCORES,
                   num_swdge_queues=max(_NQ["n"], _GMODE["nq"]))
    f32 = mybir.dt.float32
    gdt = getattr(mybir.dt, _GDT["my"])
    x_ext = nc.declare_dram_parameter("x", [N, DIN], gdt, isOutput=False)
    w_ext = nc.declare_dram_parameter("wts", [NTYPES, DIN, DOUT], gdt,
                                      isOutput=False)
    icols = sum(8 * gn for (_h, _u, gn) in groups)
    idx16_ext = nc.declare_dram_parameter(
        "idx16", [P, icols], mybir.dt.int16, isOutput=False)
    gcnt_ext = nc.declare_dram_parameter(
        "gcnt", [1, len(groups)], mybir.dt.int32, isOutput=False)
    lrow_ext = nc.declare_dram_parameter("lrow", [P, T], gdt, isOutput=False)
    wn_ext = nc.declare_dram_parameter("wn", [P, T], gdt, isOutput=False)
    iota_ext = nc.declare_dram_parameter("iota", [P, P * gcap], gdt,
                                         isOutput=False)
    bias_ext = nc.declare_dram_parameter("biasrow", [1, DOUT], f32,
                                         isOutput=False)
    ones_ext = nc.declare_dram_parameter("onesrow", [1, P], f32,
                                         isOutput=False)
    out_ext = nc.declare_dram_parameter("out", [rpc, DOUT], f32,
                                        isOutput=True)

    from contextlib import ExitStack
    stack = ExitStack()

    def sb(name, shape, dt=f32):
        return stack.enter_context(nc.sbuf_tensor(name, shape, dt))

    def ps(name, shape):
        return stack.enter_context(nc.psum_tensor(name, shape, f32))

    def sem(name):
        return stack.enter_context(nc.semaphore(name))

    with nc.Block() as block, stack:
        idx16_sb = sb("idx16_sb", [P, icols], mybir.dt.int16)
        gcnt_sb = sb("gcnt_sb", [1, len(groups)], mybir.dt.int32)
        xgg = [sb(f"xgg{i}", [P, gcap, DIN], gdt) for i in range(nbuf)]
        gg_sems = [sem(f"gg_sem{i}") for i in range(nbuf)]
        # per-unit -> (group idx, pos in group); cumulative units per group
        u2g = {}
        cumg = []
        for gi, (h, u0, gn) in enumerate(groups):
            for j in range(gn):
                u2g[u0 + j] = (gi, j)
            cumg.append(u0 + gn)
        lrow_sb = sb("lrow_sb", [P, T], gdt)
        wn_sb = sb("wn_sb", [P, T], gdt)
        iota_sb = sb("iota_sb", [P, gcap, P], gdt)
        maskb = sb("maskb", [P, gcap, P], gdt)
        w_sb = sb("w_sb", [P, NTYPES * DOUT], gdt)
        bias_sb = sb("bias_sb", [1, DOUT])
        ones_sb = sb("ones_sb", [1, P])
        pmg = [sb(f"pm{i}", [P, gcap, P], gdt) for i in range(NPB)]
        aggs = [sb(f"aggs{i}", [P, 2 * NTYPES * P], gdt) for i in range(2)]
        outs = [sb(f"outs{i}", [P, DOUT]) for i in range(2)]
        psum_agg = [ps(f"psa{i}", [P, 2 * NTYPES * P]) for i in range(2)]
        psum_out = [ps(f"pso{i}", [P, DOUT]) for i in range(2)]

        init = sem("init")
        init_g = sem("init_g")
        init_v = sem("init_v")
        msem = sem("msem")
        p_sem = sem("p_sem")
        peu = sem("peu")
        pe2 = sem("pe2")
        dcp = sem("dcp")
        act_s = sem("act_s")
        out_sems = [sem(f"out_sm{i}") for i in range(2)]

        n_init = 16 * (2 + NTYPES)
        n_init_g = 32

        cum_units = {}
        blk_entries = {}
        acc = 0
        for si, (b, t, h, nu, _f, _l) in enumerate(schedule):
            acc += nu
            cum_units[b] = acc
            blk_entries.setdefault(b, []).append(si)

        @block.sync
        def _(sp):
            sp.dma_start(idx16_sb[:], idx16_ext[:]).then_inc(init_g, 16)
            sp.dma_start(gcnt_sb[:], gcnt_ext[:]).then_inc(init_g, 16)
            sp.dma_start(lrow_sb[:], lrow_ext[:]).then_inc(init_v, 16)
            sp.dma_start(wn_sb[:], wn_ext[:]).then_inc(init_v, 16)
            sp.dma_start(iota_sb[:], iota_ext[:]).then_inc(init_v, 16)
            sp.dma_start(bias_sb[:], bias_ext[:]).then_inc(init, 16)
            sp.dma_start(ones_sb[:], ones_ext[:]).then_inc(init, 16)
            for t in range(NTYPES):
                sp.dma_start(w_sb[:, t * DOUT:(t + 1) * DOUT],
                             w_ext[t]).then_inc(init, 16)
            for rep in range(reps):
                for b in range(nblk):
                    gb = rep * nblk + b
                    nrows = min(P, rpc - b * P)
                    sp.wait_ge(act_s, gb + 1)
                    sp.dma_start(out_ext[b * P:b * P + nrows, :],
                                 outs[gb % 2][:nrows, :]
                                 ).then_inc(out_sems[gb % 2], 16)

        @block.gpsimd
        def _(g):
            from concourse.library_config import mlp
            g.load_library(mlp)
            rc = g.alloc_register("rcnt")
            g.wait_ge(msem, nbuf)
            g.wait_ge(init_g, n_init_g)
            for rep in range(reps):
                off = 0
                for gi, (h, u0, gn) in enumerate(groups):
                    gg = rep * len(groups) + gi
                    if gg >= nbuf:
                        pg = gg - nbuf
                        prep, pgi = divmod(pg, len(groups))
                        g.wait_ge(peu, prep * T + cumg[pgi])
                    g.reg_load(rc, gcnt_sb[0:1, gi:gi + 1])
                    g.dma_gather(
                        xgg[gg % nbuf][:, :gn, :],
                        x_ext[h * HALF:min((h + 1) * HALF, N), :],
                        idx16_sb[:, off:off + gn * 8],
                        gn * P, rc, DIN,
                        queue_num=gi % _GMODE["nq"],
                        single_packet=_GMODE["sp"],
                    ).then_inc(gg_sems[gg % nbuf], 16)
                    off += gn * 8

        @block.vector
        def _(v):
            # zero the gather buffers once so pad slots beyond the honest
            # gather counts read as finite zeros (mlp ucode lib has no
            # Memset, so these run on DVE)
            for i in range(nbuf):
                v.memset(xgg[i][:], 0).then_inc(msem, 1)
            v.wait_ge(init_v, 48)
            for rep in range(reps):
                for gi, (h, u0, gn) in enumerate(groups):
                    gg = rep * len(groups) + gi
                    if gg >= NPB:
                        pg = gg - NPB
                        prep, pgi = divmod(pg, len(groups))
                        v.wait_ge(peu, prep * T + cumg[pgi])
                    lb = lrow_sb[:, u0:u0 + gn].unsqueeze(2)\
                        .broadcast_to([P, gn, P])
                    wb = wn_sb[:, u0:u0 + gn].unsqueeze(2)\
                        .broadcast_to([P, gn, P])
                    v.tensor_tensor(
                        out=maskb[:, :gn, :], in0=iota_sb[:, :gn, :],
                        in1=lb, op=mybir.AluOpType.is_equal)
                    v.tensor_tensor(
                        out=pmg[gg % NPB][:, :gn, :], in0=maskb[:, :gn, :],
                        in1=wb, op=mybir.AluOpType.mult).then_inc(p_sem, gn)

        def _ph2(pe, gb):
            b = gb % nblk
            if gb == 0:
                pe.wait_ge(init, n_init)
            pe.wait_ge(dcp, gb + 1)
            if gb >= 2:
                pe.wait_ge(act_s, gb - 1)
            for k, si in enumerate(blk_entries[b]):
                _bb, t, h, _nu, _f, _l = schedule[si]
                sl = (t * 2 + h) * P
                pe.matmul(
                    out=psum_out[gb % 2][:],
                    lhsT=aggs[gb % 2][:, sl:sl + P],
                    rhs=w_sb[:, t * DOUT:(t + 1) * DOUT],
                    start=(k == 0), stop=False,
                )
            pe.matmul(out=psum_out[gb % 2][:], lhsT=ones_sb[:],
                      rhs=bias_sb[:], start=False, stop=True,
                      ).then_inc(pe2, 1)

        @block.tensor
        def _(pe):
            for rep in range(reps):
                u = 0
                for si, (b, t, h, nu, first_bt, last_bt) in enumerate(schedule):
                    gb = rep * nblk + b
                    for j in range(nu):
                        gu = rep * T + u
                        if si == blk_entries[b][0] and j == 0 and gb >= 2:
                            pe.wait_ge(dcp, gb - 1)
                        gi, jg = u2g[u]
                        gg = rep * len(groups) + gi
                        if jg == 0:
                            pe.wait_ge(gg_sems[gg % nbuf],
                                       16 * (gg // nbuf + 1))
                            pe.wait_ge(p_sem, rep * T + cumg[gi])
                        lhs = xgg[gg % nbuf][:, jg, :]
                        sl = (t * 2 + h) * P
                        pe.matmul(
                            out=psum_agg[gb % 2][:, sl:sl + P],
                            lhsT=lhs, rhs=pmg[gg % NPB][:, jg, :],
                            start=(j == 0),
                            stop=(j == nu - 1),
                        ).then_inc(peu, 1)
                        u += 1
                    if si == blk_entries[b][-1] and gb >= 1:
                        _ph2(pe, gb - 1)
            _ph2(pe, reps * nblk - 1)

        def _lrelu(act, gb):
            act.wait_ge(pe2, gb + 1)
            if gb >= 2:
                act.wait_ge(out_sems[gb % 2], 16 * ((gb - 2) // 2 + 1))
            act.activation(
                out=outs[gb % 2][:], in_=psum_out[gb % 2][:],
                func=getattr(mybir.ActivationFunctionType, _ACT["func"]),
                alpha=0.01,
            ).then_inc(act_s, 1)

        @block.scalar
        def _(act):
            for rep in range(reps):
                for b in range(nblk):
                    gb = rep * nblk + b
                    act.wait_ge(peu, rep * T + cum_units[b])
                    if gb >= 2:
                        act.wait_ge(pe2, gb - 1)
                    act.activation(
                        out=aggs[gb % 2][:], in_=psum_agg[gb % 2][:],
                        func=mybir.ActivationFunctionType.Copy,
                    ).then_inc(dcp, 1)
                    if gb >= 1:
                        _lrelu(act, gb - 1)
            _lrelu(act, reps * nblk - 1)

    nc.compile()
    return nc


def _make_in_maps(x, weights, bias_np, gidx, lrow_t, w_t,
                  schedule=None, cnts_u=None):
    gdt_np = getattr(np, _GDT["np"])
    gcap = _GMODE["gcap"]
    iota = np.tile(np.arange(P, dtype=gdt_np)[None, None, :],
                   (P, gcap, 1)).reshape(P, P * gcap)
    in_maps = []
    xg_np = x.astype(gdt_np)
    groups = _gather_groups(schedule, _GMODE["gcap"])
    idx16, gcnt = _pack_idx16(schedule, groups, gidx, cnts_u)
    for c in range(NCORES):
        in_maps.append({
            "x": xg_np,
            "wts": weights.astype(gdt_np),
            "lrow": lrow_t[c].astype(gdt_np),
            "wn": w_t[c].astype(gdt_np),
            "iota": iota,
            "biasrow": bias_np.reshape(1, DOUT),
            "onesrow": np.ones((1, P), dtype=np.float32),
            "idx16": idx16[c],
            "gcnt": gcnt[c],
        })
    return in_maps


# ---------------------------------------------------------------------------
def kernel(x, edge_index, edge_type, edge_weight, weights, bias):
    _install_sync_split()
    from concourse.bass_utils import run_bass_kernel_spmd

    x = np.asarray(x, dtype=np.float32)
    weights = np.asarray(weights, dtype=np.float32)
    bias_np = np.asarray(bias, dtype=np.float32)
    _DIMS["N"] = x.shape[0]

    schedule, T, gidx, lrow_t, w_t, cnts_u = _prepare(
        edge_index, edge_type, edge_weight)
    nc = _build_nc(schedule, T)
    in_maps = _make_in_maps(x, weights, bias_np, gidx, lrow_t, w_t,
                            schedule, cnts_u)
    res = run_bass_kernel_spmd(nc, in_maps, list(range(NCORES)))
    out = np.concatenate([res.results[c]["out"] for c in range(NCORES)],
                         axis=0)
    return out.astype(np.float32)
